# revision 38
# baseline (speedup 1.0000x reference)
"""GRU decoder with tied-embedding projection on 8 Trainium2 NeuronCores.

Problem: B=32, T=256, H=1024, V=32000 (fp32).
    h_t = GRUCell(x_t, h_{t-1});  scores_t = h_t @ emb_w.T;  x_{t+1} = emb_w[gold_t]

Sharding: vocab-parallel (column-parallel tied projection). Every core runs the
(cheap, serial) GRU recurrence redundantly; each core computes a V/8 = 4000-wide
slice of the logits. No collectives; host concatenates the vocab slices.

Because decoding is teacher-forced, the input-to-hidden activations
GI = emb_w[gold] @ w_ih.T + biases are a pure function of the inputs; they are
prepared host-side (exact fp32, like the embedding gather itself) and streamed
to the device in the gate-permuted stacked layout the recurrence consumes.

Per-core kernel structure (matmuls bf16 with fp32 PSUM accumulation):
  One fused loop over steps. Per step, PE emission order is
      [gi/bias PSUM injects][gh k-tiles][projection filler][h' transpose]
  so the projection matmuls of the previous chunk stream through the PE while
  the serial gate-math (DVE/ACT) chain runs — the PE never idles long enough
  for the HAM clock gate to re-throttle, and the chain is hidden.
  - The recurrence matmul gh = h @ w_hh.T has only B=32 output rows, so it
    uses 4-way PE *column tiling*: column group j computes a 768-wide slice
    of the (permuted) gate dim into PSUM partitions [32j, 32j+32).
  - Gate permutation P: group j holds [r,z,n] gates of hidden units
    Uj = [256j, 256j+256), so all gate math is partition-local.
  - gi_rz / the h_n bias are injected into PSUM with identity-matmuls (the
    PE array is the only cross-partition data path), removing them from the
    serial vector chain.
  - All gate math runs in bf16 (2x DVE throughput); h'^T (the next step's
    stationary operand and the projection's lhsT) is produced with
    identity-rhs matmuls.
  - Logits are staged and stored bf16 (the host widens to fp32).
"""

import sys

import numpy as np

try:
    import concourse.bass as bass  # noqa: F401
except ImportError:  # grading env may not have it on sys.path
    sys.path.insert(0, "/opt/trn_rl_repo")

import concourse.bass as bass
import concourse.tile as tile
from concourse import mybir
from concourse.bass_utils import run_bass_kernel_spmd

import ml_dtypes

BF16 = mybir.dt.bfloat16
F32 = mybir.dt.float32
AF = mybir.ActivationFunctionType
ALU = mybir.AluOpType

N_CORES = 8
B = 32
H = 1024
NK = H // 128  # 8 k-tiles over the hidden dim
G3 = 3 * H     # 3072 gates


def _split_multi_waits(nc, limit=1):
    """Walrus (CoreV3, public build) accepts at most `limit` sem waits per
    instruction; move extra waits onto NoOps inserted just before."""
    n_new = 0
    for _name, bbw in nc.bb_map.items():
        insts = bbw.bb.instructions
        out, changed = [], False
        for inst in insts:
            si = inst.sync_info
            ws = list(si.on_wait) if si is not None else []
            if len(ws) > limit:
                changed = True
                for i in range(limit, len(ws), limit):
                    n_new += 1
                    nop = mybir.InstNoOp(
                        name=f"I-wsplit-{n_new}", engine=inst.engine, ins=[], outs=[]
                    )
                    nop.sync_info = mybir.SyncInfo(on_wait=ws[i : i + limit], on_update=[])
                    out.append(nop)
                inst.sync_info = mybir.SyncInfo(
                    on_wait=ws[:limit], on_update=list(si.on_update)
                )
            out.append(inst)
        if changed:
            bbw.bb.instructions = out
    return n_new


def _gate_perm():
    """P such that permuted gate column g' = 768j + {0:r,256:z,512:n} + i maps
    to original gate row P[g'] of w_ih / w_hh (PyTorch order r|z|n)."""
    P = np.empty(G3, np.int64)
    for j in range(4):
        u = np.arange(256) + 256 * j
        P[768 * j : 768 * j + 256] = u
        P[768 * j + 256 : 768 * j + 512] = H + u
        P[768 * j + 512 : 768 * j + 768] = 2 * H + u
    return P


def _kblock(a):
    """[H, X] -> [128, NK*X]  (k-tile k occupies columns [k*X, (k+1)*X))."""
    hh, x = a.shape
    assert hh == H
    return np.ascontiguousarray(a.reshape(NK, 128, x).transpose(1, 0, 2).reshape(128, NK * x))


def _bf16(a):
    return np.asarray(a, dtype=ml_dtypes.bfloat16)


def build_program(T, Vs, Tc):
    """Build the SPMD bass program (identical on all cores)."""
    assert T % Tc == 0 and Tc % 4 == 0
    NCH = T // Tc            # chunks
    NV = Vs // 500           # 500-wide vocab chunks
    NM = (Tc * B) // 128     # projection m-tiles per chunk

    nc = bass.Bass()
    d_whh = nc.declare_dram_parameter("whhp", [128, NK * G3], BF16, isOutput=False)
    d_emb = nc.declare_dram_parameter("embc", [128, NK * Vs], BF16, isOutput=False)
    d_gi = nc.declare_dram_parameter("gistk", [T, 128, 768], BF16, isOutput=False)
    d_bhn = nc.declare_dram_parameter("bhhn", [128, H], BF16, isOutput=False)
    d_i128 = nc.declare_dram_parameter("i128", [128, 128], BF16, isOutput=False)
    d_h0b = nc.declare_dram_parameter("h0b", [128, 256], BF16, isOutput=False)
    d_h0t = nc.declare_dram_parameter("h0t", [128, 256], BF16, isOutput=False)
    d_out = nc.declare_dram_parameter("scores", [B, T, Vs], BF16, isOutput=True)

    with tile.TileContext(nc) as tc:
        with (
            tc.tile_pool(name="res", bufs=1) as res,
            tc.tile_pool(name="consts", bufs=1) as consts,
            tc.tile_pool(name="gistep", bufs=6) as p_gi,
            tc.tile_pool(name="ht", bufs=2) as p_ht,
            tc.tile_pool(name="gates", bufs=2) as p_gates,
            tc.tile_pool(name="hb", bufs=2) as p_hb,
            tc.tile_pool(name="hstrip", bufs=2) as p_hs,
            tc.tile_pool(name="pstage", bufs=6) as p_stage,
            tc.tile_pool(name="psgh", bufs=1, space="PSUM") as p_psgh,
            tc.tile_pool(name="psht", bufs=2, space="PSUM") as p_psht,
            tc.tile_pool(name="pspr", bufs=4, space="PSUM") as p_pspr,
        ):
            # Startup uploads are chained with 1-elem WAW gates so the bytes
            # the kernel needs first get the full DMA bandwidth: whh k0 ->
            # k1 -> ... -> k7 -> emb n0 -> n1 -> ... (step 0 needs whh k0;
            # the first projection filler needs emb n-chunks in order).
            # Depth-3 pipelined gating: chunk i waits chunk i-3, so ~3 uploads
            # are in flight at all times — arrival order is preserved (the
            # bytes step 0 needs come first) while the per-hop semaphore
            # latency that starved the serial chain is hidden.
            whh = res.tile([128, NK * G3], BF16, tag="whh")
            for k in range(NK):
                if k >= 3:
                    nc.gpsimd.tensor_copy(
                        whh[0:1, k * G3 : k * G3 + 1],
                        whh[0:1, (k - 3) * G3 : (k - 3) * G3 + 1],
                    )
                nc.sync.dma_start(
                    whh[:, k * G3 : (k + 1) * G3], d_whh[:, k * G3 : (k + 1) * G3]
                )
            # emb is laid out n-major (chunk n holds its 8 k-slices
            # contiguously), so each chunk upload is one contiguous DMA
            emb = res.tile([128, NK * Vs], BF16, tag="emb")
            NCW = NK * 500  # columns per n-chunk
            for n in range(NV):
                gate_src = (
                    whh[0:1, (NK - 3 + n) * G3 : (NK - 3 + n) * G3 + 1]
                    if n < 3
                    else emb[0:1, (n - 3) * NCW : (n - 3) * NCW + 1]
                )
                nc.gpsimd.tensor_copy(emb[0:1, n * NCW : n * NCW + 1], gate_src)
                nc.sync.dma_start(
                    emb[:, n * NCW : (n + 1) * NCW],
                    d_emb[:, n * NCW : (n + 1) * NCW],
                )
            bhn = consts.tile([128, H], BF16, tag="bhn")
            nc.sync.dma_start(bhn[:], d_bhn[:])
            i128 = consts.tile([128, 128], BF16, tag="i128")
            nc.sync.dma_start(i128[:], d_i128[:])
            h0b = consts.tile([128, 256], BF16, tag="h0b")
            nc.sync.dma_start(h0b[:], d_h0b[:])
            h0t = consts.tile([128, 256], BF16, tag="h0t")
            nc.sync.dma_start(h0t[:], d_h0t[:])

            h_prev = h0b[:]           # b-major: partition 32j+b, cols = units of Uj
            ht_prev = h0t[:]          # unit-major cols: 32*(4h+j)+b  (k = 2j+h)
            proj_queue = []           # pending (ci, ht_view, m, n) projection units

            def emit_proj_mms(ts_c, Tcc, ht_v, m, n, k_hi=None, pp=None, k_lo=0):
                """PE part of one projection unit; drain is emitted later so
                the strict-FIFO DVE/ACT queues never block the gate chain."""
                if pp is None:
                    pp = p_pspr.tile([128, 500], F32, tag="pspr")
                if k_hi is None:
                    k_hi = NK
                for k in range(k_lo, k_hi):
                    nc.tensor.matmul(
                        pp[:],
                        ht_v[:, k, m * 128 : m * 128 + 128],
                        emb[:, (n * NK + k) * 500 : (n * NK + k) * 500 + 500],
                        start=(k == 0),
                        stop=(k == NK - 1),
                    )
                return (ts_c, Tcc, m, n, pp)

            def emit_proj_drain(ts_c, Tcc, m, n, pp, eng):
                bpm = (128 * m) // Tcc     # first batch row of m-tile
                nb = 128 // Tcc            # batch rows per m-tile
                st = p_stage.tile([128, 500], BF16, tag="pstage")
                # alternate the PSUM->SBUF copy between ACT and DVE
                if eng % 2 == 0:
                    nc.scalar.copy(st[:], pp[:])
                else:
                    nc.vector.tensor_copy(st[:], pp[:])
                nc.sync.dma_start(
                    d_out[
                        bpm : bpm + nb,
                        ts_c : ts_c + Tcc,
                        n * 500 : n * 500 + 500,
                    ],
                    st[:],
                )

            def queue_projection(ts_c, Tcc, ht_c):
                ht_v = ht_c[:].rearrange("p (k c) -> p k c", k=NK)
                for m in range((Tcc * B) // 128):
                    for n in range(NV):
                        proj_queue.append((ts_c, Tcc, ht_v, m, n))

            PPS = -(-(NM * NV) // Tc)  # proj units to emit per step

            # short chunks at the start (projection filler becomes available
            # sooner) and at the end (smaller post-loop drain tail)
            chunks = [4, 4] + [8] * ((T - 16) // 8) + [4, 4]
            assert sum(chunks) == T
            t = 0
            for Tcc in chunks:
                ts_c = t
                # HT chunk: col = k*(B*Tcc) + b*Tcc + tl  (k = 2j+h)
                ht_c = p_ht.tile([128, NK * B * Tcc], BF16, tag="ht")
                for tl in range(Tcc):
                    t = ts_c + tl
                    # gi for this step, host-prepared in stacked layout:
                    # partition 32j+b, cols 768j:768j+768 of the permuted gates
                    gi_s = p_gi.tile([128, 768], BF16, tag="gistep")
                    nc.sync.dma_start(gi_s[:], d_gi[t])
                    gh = p_psgh.tile([128, 768], F32, tag="psgh")
                    # --- PSUM injects: gi_rz starts the rz bank, bhn the n bank
                    for j in range(4):
                        nc.tensor.matmul(
                            gh[32 * j : 32 * j + 32, 0:512],
                            i128[:, 32 * j : 32 * j + 32],
                            gi_s[:, 0:512],
                            start=True,
                            stop=False,
                            tile_position=(0, 32 * j),
                        )
                    for j in range(4):
                        nc.tensor.matmul(
                            gh[32 * j : 32 * j + 32, 512:768],
                            i128[:, 0:32],
                            bhn[:, 256 * j : 256 * j + 256],
                            start=True,
                            stop=False,
                            tile_position=(0, 32 * j),
                        )
                    # --- recurrence matmuls, col-tiled 4 ways ---
                    # (all 4 col-groups' 512-MMs adjacent, then the 256-MMs:
                    # MM starts are pc-monotone, so same-group back-to-back
                    # MMs would serialize the group concurrency)
                    for k in range(NK):
                        pos = (k % 2) * 4 + k // 2
                        lhs = ht_prev[:, 32 * pos : 32 * pos + 32]
                        for j in range(4):
                            nc.tensor.matmul(
                                gh[32 * j : 32 * j + 32, 0:512],
                                lhs,
                                whh[:, k * G3 + 768 * j : k * G3 + 768 * j + 512],
                                start=False,
                                stop=(k == NK - 1),
                                tile_position=(0, 32 * j),
                            )
                        for j in range(4):
                            nc.tensor.matmul(
                                gh[32 * j : 32 * j + 32, 512:768],
                                lhs,
                                whh[:, k * G3 + 768 * j + 512 : k * G3 + 768 * j + 768],
                                start=False,
                                stop=(k == NK - 1),
                                tile_position=(0, 32 * j),
                            )
                    # --- projection filler: streams through the PE while the
                    # serial gate-math chain below runs on DVE/ACT. The last
                    # unit's final k-MMs are held back until after the h'
                    # transpose so they cover the h-strip copy window. ---
                    pending = []
                    held = None
                    HB = 2  # held-back k-MMs of the last unit
                    nun = min(PPS, len(proj_queue))
                    for ui in range(nun):
                        if ui < nun - 1:
                            pending.append(emit_proj_mms(*proj_queue.pop(0)))
                        else:
                            held = proj_queue.pop(0)
                            pending.append(emit_proj_mms(*held, k_hi=NK - HB))
                    # --- gate math (bf16), g cols:
                    # 0:512 rz | 512:768 t1 | 768:1024 t2 | 1024:1280 n
                    # 1280:1536 d=h-n | 1536:1792 s=z*d;  h' = n + s
                    # (every DVE op depends on the previous one, so the
                    # readiness-driven scheduler cannot reorder the chain)
                    g = p_gates.tile([128, 1792], BF16, tag="gates")
                    nc.scalar.activation(g[:, 0:512], gh[:, 0:512], AF.Sigmoid)
                    nc.vector.tensor_tensor(
                        g[:, 512:768], g[:, 0:256], gh[:, 512:768], ALU.mult
                    )
                    nc.vector.tensor_tensor(
                        g[:, 768:1024], g[:, 512:768], gi_s[:, 512:768], ALU.add
                    )
                    nc.scalar.activation(g[:, 1024:1280], g[:, 768:1024], AF.Tanh)
                    nc.vector.tensor_tensor(
                        g[:, 1280:1536], h_prev, g[:, 1024:1280], ALU.subtract
                    )
                    nc.vector.tensor_tensor(
                        g[:, 1536:1792], g[:, 256:512], g[:, 1280:1536], ALU.mult
                    )
                    hb = p_hb.tile([128, 256], BF16, tag="hb")
                    nc.vector.tensor_tensor(
                        hb[:], g[:, 1024:1280], g[:, 1536:1792], ALU.add
                    )
                    # --- transpose h' via identity-rhs matmuls ---
                    pT = p_psht.tile([128, 256], F32, tag="psht")
                    nc.tensor.matmul(
                        pT[:, 0:128], hb[:, 0:128], i128[:], start=True, stop=True
                    )
                    nc.tensor.matmul(
                        pT[:, 128:256], hb[:, 128:256], i128[:], start=True, stop=True
                    )
                    # h-strip copy split across DVE/ACT so the next step's
                    # first LDWEIGHTS (cols 0:32) unblocks as early as possible
                    hs = p_hs.tile([128, 256], BF16, tag="hstrip")
                    nc.vector.tensor_copy(hs[:, 0:128], pT[:, 0:128])
                    nc.scalar.copy(hs[:, 128:256], pT[:, 128:256])
                    # finish the held-back projection unit under the h-copy
                    if held is not None:
                        emit_proj_mms(*held, pp=pending[-1][4], k_lo=NK - HB)
                    # scatter h^T into the chunk's HT (b-major cols b*Tcc+tl)
                    dst = ht_c[:].rearrange(
                        "p (j h b t) -> p h j b t", j=4, h=2, b=B, t=Tcc
                    )[:, :, :, :, tl]
                    src = hs[:].rearrange("p (h j b) -> p h j b", h=2, j=4)
                    nc.vector.tensor_copy(dst, src)
                    ht_prev = hs[:]
                    h_prev = hb[:]
                    # drain this step's projection PSUM (deps long satisfied)
                    for i, pu in enumerate(pending):
                        emit_proj_drain(*pu, eng=i)
                t = ts_c + Tcc
                queue_projection(ts_c, Tcc, ht_c)
            while proj_queue:
                pu = emit_proj_mms(*proj_queue.pop(0))
                emit_proj_drain(*pu, eng=pu[3])

    nc.finalize()
    _split_multi_waits(nc)
    return nc


def prep_inputs(enc_hiddens, emb_w, w_ih, w_hh, b_ih, b_hh, gold, T, Vs, n_cores):
    """Host-side shard + layout prep. Returns per-core input maps."""
    P = _gate_perm()
    h0 = np.asarray(enc_hiddens, np.float32)[0]          # [B, H]
    emb_w = np.asarray(emb_w, np.float32)
    w_ih = np.asarray(w_ih, np.float32)
    w_hh = np.asarray(w_hh, np.float32)
    b_ih = np.asarray(b_ih, np.float32)
    b_hh = np.asarray(b_hh, np.float32)
    gold = np.asarray(gold)

    whhp = _bf16(_kblock(w_hh[P].T))
    # teacher-forced inputs -> exact input-side gate activations
    idx = np.empty((T, B), np.int64)
    idx[0] = 1  # START_IDX
    if T > 1:
        idx[1:] = gold[:, : T - 1].T
    X = emb_w[idx].reshape(T * B, H)                      # [T*B, H]
    GI = X @ w_ih.T + b_ih
    GI[:, : 2 * H] += b_hh[: 2 * H]                       # rz: both biases folded
    GIp = GI[:, P]                                        # permuted gate cols
    gistk = _bf16(
        GIp.reshape(T, B, 4, 768).transpose(0, 2, 1, 3).reshape(T, 128, 768)
    )
    bhn_row = b_hh[2 * H :]                               # [H], unit u order
    bhhn = _bf16(np.broadcast_to(bhn_row, (128, H)))
    i128 = _bf16(np.eye(128, dtype=np.float32))
    # H0B[32j+b, u'] = h0[b, 256j+u']
    h0b = _bf16(
        np.ascontiguousarray(h0.reshape(B, 4, 256).transpose(1, 0, 2).reshape(128, 256))
    )
    # H0T[q, 32*(4h+j)+b] = h0[b, 256j+128h+q]
    h0t = _bf16(
        np.ascontiguousarray(h0.reshape(B, 4, 2, 128).transpose(3, 2, 1, 0).reshape(128, 256))
    )
    embT = emb_w.T                                        # [H, V]
    NV = Vs // 500
    maps = []
    for c in range(n_cores):
        embk = _kblock(np.ascontiguousarray(embT[:, c * Vs : (c + 1) * Vs]))
        # n-major: chunk n's 8 k-slices contiguous -> [p, (n k v)]
        embc = _bf16(
            np.ascontiguousarray(
                embk.reshape(128, NK, NV, 500).transpose(0, 2, 1, 3).reshape(128, NK * Vs)
            )
        )
        maps.append(
            dict(
                whhp=whhp, embc=embc, gistk=gistk, bhhn=bhhn,
                i128=i128, h0b=h0b, h0t=h0t,
            )
        )
    return maps


_CACHE = {}


def run(enc_hiddens, emb_w, w_ih, w_hh, b_ih, b_hh, gold, T, Vs, n_cores, Tc,
        trace=False):
    key = (T, Vs, n_cores, Tc)
    if key not in _CACHE:
        _CACHE[key] = build_program(T, Vs, Tc)
    nc = _CACHE[key]
    maps = prep_inputs(enc_hiddens, emb_w, w_ih, w_hh, b_ih, b_hh, gold, T, Vs, n_cores)
    res = run_bass_kernel_spmd(nc, maps, list(range(n_cores)), trace=trace)
    out = np.concatenate(
        [res.results[c]["scores"].astype(np.float32) for c in range(n_cores)], axis=2
    )
    return out, res


def kernel(enc_hiddens, emb_w, w_ih, w_hh, b_ih, b_hh, gold):
    T, Vs = 256, 32000 // N_CORES
    out, _ = run(enc_hiddens, emb_w, w_ih, w_hh, b_ih, b_hh, gold, T, Vs, N_CORES, Tc=8)
    return out


# revision 40
# speedup vs baseline: 1.0168x; 1.0168x over previous
"""GRU decoder with tied-embedding projection on 8 Trainium2 NeuronCores.

Problem: B=32, T=256, H=1024, V=32000 (fp32).
    h_t = GRUCell(x_t, h_{t-1});  scores_t = h_t @ emb_w.T;  x_{t+1} = emb_w[gold_t]

Sharding: vocab-parallel (column-parallel tied projection). Every core runs the
(cheap, serial) GRU recurrence redundantly; each core computes a V/8 = 4000-wide
slice of the logits. No collectives; host concatenates the vocab slices.

Because decoding is teacher-forced, the input-to-hidden activations
GI = emb_w[gold] @ w_ih.T + biases are a pure function of the inputs; they are
prepared host-side (exact fp32, like the embedding gather itself) and streamed
to the device in the gate-permuted stacked layout the recurrence consumes.

Per-core kernel structure (matmuls bf16 with fp32 PSUM accumulation):
  One fused loop over steps. Per step, PE emission order is
      [gi/bias PSUM injects][gh k-tiles][projection filler][h' transpose]
  so the projection matmuls of the previous chunk stream through the PE while
  the serial gate-math (DVE/ACT) chain runs — the PE never idles long enough
  for the HAM clock gate to re-throttle, and the chain is hidden.
  - The recurrence matmul gh = h @ w_hh.T has only B=32 output rows, so it
    uses 4-way PE *column tiling*: column group j computes a 768-wide slice
    of the (permuted) gate dim into PSUM partitions [32j, 32j+32).
  - Gate permutation P: group j holds [r,z,n] gates of hidden units
    Uj = [256j, 256j+256), so all gate math is partition-local.
  - gi_rz / the h_n bias are injected into PSUM with identity-matmuls (the
    PE array is the only cross-partition data path), removing them from the
    serial vector chain.
  - All gate math runs in bf16 (2x DVE throughput); h'^T (the next step's
    stationary operand and the projection's lhsT) is produced with
    identity-rhs matmuls.
  - Logits are staged and stored bf16 (the host widens to fp32).
"""

import sys

import numpy as np

try:
    import concourse.bass as bass  # noqa: F401
except ImportError:  # grading env may not have it on sys.path
    sys.path.insert(0, "/opt/trn_rl_repo")

import concourse.bass as bass
import concourse.tile as tile
from concourse import mybir
from concourse.bass_utils import run_bass_kernel_spmd

import ml_dtypes

BF16 = mybir.dt.bfloat16
F32 = mybir.dt.float32
AF = mybir.ActivationFunctionType
ALU = mybir.AluOpType

N_CORES = 8
B = 32
H = 1024
NK = H // 128  # 8 k-tiles over the hidden dim
G3 = 3 * H     # 3072 gates


def _split_multi_waits(nc, limit=1):
    """Walrus (CoreV3, public build) accepts at most `limit` sem waits per
    instruction; move extra waits onto NoOps inserted just before."""
    n_new = 0
    for _name, bbw in nc.bb_map.items():
        insts = bbw.bb.instructions
        out, changed = [], False
        for inst in insts:
            si = inst.sync_info
            ws = list(si.on_wait) if si is not None else []
            if len(ws) > limit:
                changed = True
                for i in range(limit, len(ws), limit):
                    n_new += 1
                    nop = mybir.InstNoOp(
                        name=f"I-wsplit-{n_new}", engine=inst.engine, ins=[], outs=[]
                    )
                    nop.sync_info = mybir.SyncInfo(on_wait=ws[i : i + limit], on_update=[])
                    out.append(nop)
                inst.sync_info = mybir.SyncInfo(
                    on_wait=ws[:limit], on_update=list(si.on_update)
                )
            out.append(inst)
        if changed:
            bbw.bb.instructions = out
    return n_new


def _gate_perm():
    """P such that permuted gate column g' = 768j + {0:r,256:z,512:n} + i maps
    to original gate row P[g'] of w_ih / w_hh (PyTorch order r|z|n)."""
    P = np.empty(G3, np.int64)
    for j in range(4):
        u = np.arange(256) + 256 * j
        P[768 * j : 768 * j + 256] = u
        P[768 * j + 256 : 768 * j + 512] = H + u
        P[768 * j + 512 : 768 * j + 768] = 2 * H + u
    return P


def _kblock(a):
    """[H, X] -> [128, NK*X]  (k-tile k occupies columns [k*X, (k+1)*X))."""
    hh, x = a.shape
    assert hh == H
    return np.ascontiguousarray(a.reshape(NK, 128, x).transpose(1, 0, 2).reshape(128, NK * x))


def _bf16(a):
    return np.asarray(a, dtype=ml_dtypes.bfloat16)


def build_program(T, Vs, Tc):
    """Build the SPMD bass program (identical on all cores)."""
    assert T % Tc == 0 and Tc % 4 == 0
    NCH = T // Tc            # chunks
    NV = Vs // 500           # 500-wide vocab chunks
    NM = (Tc * B) // 128     # projection m-tiles per chunk

    nc = bass.Bass()
    d_whh = nc.declare_dram_parameter("whhp", [128, NK * G3], BF16, isOutput=False)
    d_emb = nc.declare_dram_parameter("embc", [128, NK * Vs], BF16, isOutput=False)
    d_gi = nc.declare_dram_parameter("gistk", [T, 128, 768], BF16, isOutput=False)
    d_bhn = nc.declare_dram_parameter("bhhn", [128, H], BF16, isOutput=False)
    d_i128 = nc.declare_dram_parameter("i128", [128, 128], BF16, isOutput=False)
    d_h0b = nc.declare_dram_parameter("h0b", [128, 256], BF16, isOutput=False)
    d_h0t = nc.declare_dram_parameter("h0t", [128, 256], BF16, isOutput=False)
    d_out = nc.declare_dram_parameter("scores", [B, T, Vs], BF16, isOutput=True)

    with tile.TileContext(nc) as tc:
        with (
            tc.tile_pool(name="res", bufs=1) as res,
            tc.tile_pool(name="consts", bufs=1) as consts,
            tc.tile_pool(name="gistep", bufs=6) as p_gi,
            tc.tile_pool(name="ht", bufs=2) as p_ht,
            tc.tile_pool(name="gates", bufs=2) as p_gates,
            tc.tile_pool(name="hb", bufs=2) as p_hb,
            tc.tile_pool(name="hstrip", bufs=2) as p_hs,
            tc.tile_pool(name="pstage", bufs=6) as p_stage,
            tc.tile_pool(name="psgh", bufs=1, space="PSUM") as p_psgh,
            tc.tile_pool(name="psht", bufs=2, space="PSUM") as p_psht,
            tc.tile_pool(name="pspr", bufs=4, space="PSUM") as p_pspr,
        ):
            # Startup uploads are chained with 1-elem WAW gates so the bytes
            # the kernel needs first get the full DMA bandwidth: whh k0 ->
            # k1 -> ... -> k7 -> emb n0 -> n1 -> ... (step 0 needs whh k0;
            # the first projection filler needs emb n-chunks in order).
            # Depth-3 pipelined gating: chunk i waits chunk i-3, so ~3 uploads
            # are in flight at all times — arrival order is preserved (the
            # bytes step 0 needs come first) while the per-hop semaphore
            # latency that starved the serial chain is hidden.
            whh = res.tile([128, NK * G3], BF16, tag="whh")
            for k in range(NK):
                if k >= 3:
                    nc.gpsimd.tensor_copy(
                        whh[0:1, k * G3 : k * G3 + 1],
                        whh[0:1, (k - 3) * G3 : (k - 3) * G3 + 1],
                    )
                nc.gpsimd.dma_start(
                    whh[:, k * G3 : (k + 1) * G3], d_whh[:, k * G3 : (k + 1) * G3]
                )
            # emb is laid out n-major (chunk n holds its 8 k-slices
            # contiguously), so each chunk upload is one contiguous DMA
            emb = res.tile([128, NK * Vs], BF16, tag="emb")
            NCW = NK * 500  # columns per n-chunk
            for n in range(NV):
                gate_src = (
                    whh[0:1, (NK - 3 + n) * G3 : (NK - 3 + n) * G3 + 1]
                    if n < 3
                    else emb[0:1, (n - 3) * NCW : (n - 3) * NCW + 1]
                )
                nc.gpsimd.tensor_copy(emb[0:1, n * NCW : n * NCW + 1], gate_src)
                nc.gpsimd.dma_start(
                    emb[:, n * NCW : (n + 1) * NCW],
                    d_emb[:, n * NCW : (n + 1) * NCW],
                )
            bhn = consts.tile([128, H], BF16, tag="bhn")
            nc.sync.dma_start(bhn[:], d_bhn[:])
            i128 = consts.tile([128, 128], BF16, tag="i128")
            nc.sync.dma_start(i128[:], d_i128[:])
            h0b = consts.tile([128, 256], BF16, tag="h0b")
            nc.sync.dma_start(h0b[:], d_h0b[:])
            h0t = consts.tile([128, 256], BF16, tag="h0t")
            nc.sync.dma_start(h0t[:], d_h0t[:])

            h_prev = h0b[:]           # b-major: partition 32j+b, cols = units of Uj
            ht_prev = h0t[:]          # unit-major cols: 32*(4h+j)+b  (k = 2j+h)
            proj_queue = []           # pending (ci, ht_view, m, n) projection units

            def emit_proj_mms(ts_c, Tcc, ht_v, m, n, k_hi=None, pp=None, k_lo=0):
                """PE part of one projection unit; drain is emitted later so
                the strict-FIFO DVE/ACT queues never block the gate chain."""
                if pp is None:
                    pp = p_pspr.tile([128, 500], F32, tag="pspr")
                if k_hi is None:
                    k_hi = NK
                for k in range(k_lo, k_hi):
                    nc.tensor.matmul(
                        pp[:],
                        ht_v[:, k, m * 128 : m * 128 + 128],
                        emb[:, (n * NK + k) * 500 : (n * NK + k) * 500 + 500],
                        start=(k == 0),
                        stop=(k == NK - 1),
                    )
                return (ts_c, Tcc, m, n, pp)

            def emit_proj_drain(ts_c, Tcc, m, n, pp, eng):
                bpm = (128 * m) // Tcc     # first batch row of m-tile
                nb = 128 // Tcc            # batch rows per m-tile
                st = p_stage.tile([128, 500], BF16, tag="pstage")
                # alternate the PSUM->SBUF copy between ACT and DVE
                if eng % 2 == 0:
                    nc.scalar.copy(st[:], pp[:])
                else:
                    nc.vector.tensor_copy(st[:], pp[:])
                nc.sync.dma_start(
                    d_out[
                        bpm : bpm + nb,
                        ts_c : ts_c + Tcc,
                        n * 500 : n * 500 + 500,
                    ],
                    st[:],
                )

            def queue_projection(ts_c, Tcc, ht_c):
                ht_v = ht_c[:].rearrange("p (k c) -> p k c", k=NK)
                for m in range((Tcc * B) // 128):
                    for n in range(NV):
                        proj_queue.append((ts_c, Tcc, ht_v, m, n))

            PPS = -(-(NM * NV) // Tc)  # proj units to emit per step

            # short chunks at the start (projection filler becomes available
            # sooner) and at the end (smaller post-loop drain tail)
            chunks = [4, 4] + [8] * ((T - 16) // 8) + [4, 4]
            assert sum(chunks) == T
            t = 0
            for Tcc in chunks:
                ts_c = t
                # HT chunk: col = k*(B*Tcc) + b*Tcc + tl  (k = 2j+h)
                ht_c = p_ht.tile([128, NK * B * Tcc], BF16, tag="ht")
                for tl in range(Tcc):
                    t = ts_c + tl
                    # gi for this step, host-prepared in stacked layout:
                    # partition 32j+b, cols 768j:768j+768 of the permuted gates
                    gi_s = p_gi.tile([128, 768], BF16, tag="gistep")
                    nc.sync.dma_start(gi_s[:], d_gi[t])
                    gh = p_psgh.tile([128, 768], F32, tag="psgh")
                    # --- PSUM injects: gi_rz starts the rz bank, bhn the n bank
                    for j in range(4):
                        nc.tensor.matmul(
                            gh[32 * j : 32 * j + 32, 0:512],
                            i128[:, 32 * j : 32 * j + 32],
                            gi_s[:, 0:512],
                            start=True,
                            stop=False,
                            tile_position=(0, 32 * j),
                        )
                    for j in range(4):
                        nc.tensor.matmul(
                            gh[32 * j : 32 * j + 32, 512:768],
                            i128[:, 0:32],
                            bhn[:, 256 * j : 256 * j + 256],
                            start=True,
                            stop=False,
                            tile_position=(0, 32 * j),
                        )
                    # --- recurrence matmuls, col-tiled 4 ways ---
                    # (all 4 col-groups' 512-MMs adjacent, then the 256-MMs:
                    # MM starts are pc-monotone, so same-group back-to-back
                    # MMs would serialize the group concurrency)
                    for k in range(NK):
                        pos = (k % 2) * 4 + k // 2
                        lhs = ht_prev[:, 32 * pos : 32 * pos + 32]
                        for j in range(4):
                            nc.tensor.matmul(
                                gh[32 * j : 32 * j + 32, 0:512],
                                lhs,
                                whh[:, k * G3 + 768 * j : k * G3 + 768 * j + 512],
                                start=False,
                                stop=(k == NK - 1),
                                tile_position=(0, 32 * j),
                            )
                        for j in range(4):
                            nc.tensor.matmul(
                                gh[32 * j : 32 * j + 32, 512:768],
                                lhs,
                                whh[:, k * G3 + 768 * j + 512 : k * G3 + 768 * j + 768],
                                start=False,
                                stop=(k == NK - 1),
                                tile_position=(0, 32 * j),
                            )
                    # --- projection filler: streams through the PE while the
                    # serial gate-math chain below runs on DVE/ACT. The last
                    # unit's final k-MMs are held back until after the h'
                    # transpose so they cover the h-strip copy window. ---
                    pending = []
                    held = None
                    HB = 2  # held-back k-MMs of the last unit
                    nun = min(PPS, len(proj_queue))
                    for ui in range(nun):
                        if ui < nun - 1:
                            pending.append(emit_proj_mms(*proj_queue.pop(0)))
                        else:
                            held = proj_queue.pop(0)
                            pending.append(emit_proj_mms(*held, k_hi=NK - HB))
                    # --- gate math (bf16), g cols:
                    # 0:512 rz | 512:768 t1 | 768:1024 t2 | 1024:1280 n
                    # 1280:1536 d=h-n | 1536:1792 s=z*d;  h' = n + s
                    # (every DVE op depends on the previous one, so the
                    # readiness-driven scheduler cannot reorder the chain)
                    g = p_gates.tile([128, 1792], BF16, tag="gates")
                    nc.scalar.activation(g[:, 0:512], gh[:, 0:512], AF.Sigmoid)
                    nc.vector.tensor_tensor(
                        g[:, 512:768], g[:, 0:256], gh[:, 512:768], ALU.mult
                    )
                    nc.vector.tensor_tensor(
                        g[:, 768:1024], g[:, 512:768], gi_s[:, 512:768], ALU.add
                    )
                    nc.scalar.activation(g[:, 1024:1280], g[:, 768:1024], AF.Tanh)
                    nc.vector.tensor_tensor(
                        g[:, 1280:1536], h_prev, g[:, 1024:1280], ALU.subtract
                    )
                    nc.vector.tensor_tensor(
                        g[:, 1536:1792], g[:, 256:512], g[:, 1280:1536], ALU.mult
                    )
                    hb = p_hb.tile([128, 256], BF16, tag="hb")
                    nc.vector.tensor_tensor(
                        hb[:], g[:, 1024:1280], g[:, 1536:1792], ALU.add
                    )
                    # --- transpose h' via identity-rhs matmuls ---
                    pT = p_psht.tile([128, 256], F32, tag="psht")
                    nc.tensor.matmul(
                        pT[:, 0:128], hb[:, 0:128], i128[:], start=True, stop=True
                    )
                    nc.tensor.matmul(
                        pT[:, 128:256], hb[:, 128:256], i128[:], start=True, stop=True
                    )
                    # h-strip copy split across DVE/ACT so the next step's
                    # first LDWEIGHTS (cols 0:32) unblocks as early as possible
                    hs = p_hs.tile([128, 256], BF16, tag="hstrip")
                    nc.vector.tensor_copy(hs[:, 0:128], pT[:, 0:128])
                    nc.scalar.copy(hs[:, 128:256], pT[:, 128:256])
                    # finish the held-back projection unit under the h-copy
                    if held is not None:
                        emit_proj_mms(*held, pp=pending[-1][4], k_lo=NK - HB)
                    # scatter h^T into the chunk's HT (b-major cols b*Tcc+tl)
                    dst = ht_c[:].rearrange(
                        "p (j h b t) -> p h j b t", j=4, h=2, b=B, t=Tcc
                    )[:, :, :, :, tl]
                    src = hs[:].rearrange("p (h j b) -> p h j b", h=2, j=4)
                    nc.vector.tensor_copy(dst, src)
                    ht_prev = hs[:]
                    h_prev = hb[:]
                    # drain this step's projection PSUM (deps long satisfied)
                    for i, pu in enumerate(pending):
                        emit_proj_drain(*pu, eng=i)
                t = ts_c + Tcc
                queue_projection(ts_c, Tcc, ht_c)
            while proj_queue:
                pu = emit_proj_mms(*proj_queue.pop(0))
                emit_proj_drain(*pu, eng=pu[3])

    nc.finalize()
    _split_multi_waits(nc)
    return nc


def prep_inputs(enc_hiddens, emb_w, w_ih, w_hh, b_ih, b_hh, gold, T, Vs, n_cores):
    """Host-side shard + layout prep. Returns per-core input maps."""
    P = _gate_perm()
    h0 = np.asarray(enc_hiddens, np.float32)[0]          # [B, H]
    emb_w = np.asarray(emb_w, np.float32)
    w_ih = np.asarray(w_ih, np.float32)
    w_hh = np.asarray(w_hh, np.float32)
    b_ih = np.asarray(b_ih, np.float32)
    b_hh = np.asarray(b_hh, np.float32)
    gold = np.asarray(gold)

    whhp = _bf16(_kblock(w_hh[P].T))
    # teacher-forced inputs -> exact input-side gate activations
    idx = np.empty((T, B), np.int64)
    idx[0] = 1  # START_IDX
    if T > 1:
        idx[1:] = gold[:, : T - 1].T
    X = emb_w[idx].reshape(T * B, H)                      # [T*B, H]
    GI = X @ w_ih.T + b_ih
    GI[:, : 2 * H] += b_hh[: 2 * H]                       # rz: both biases folded
    GIp = GI[:, P]                                        # permuted gate cols
    gistk = _bf16(
        GIp.reshape(T, B, 4, 768).transpose(0, 2, 1, 3).reshape(T, 128, 768)
    )
    bhn_row = b_hh[2 * H :]                               # [H], unit u order
    bhhn = _bf16(np.broadcast_to(bhn_row, (128, H)))
    i128 = _bf16(np.eye(128, dtype=np.float32))
    # H0B[32j+b, u'] = h0[b, 256j+u']
    h0b = _bf16(
        np.ascontiguousarray(h0.reshape(B, 4, 256).transpose(1, 0, 2).reshape(128, 256))
    )
    # H0T[q, 32*(4h+j)+b] = h0[b, 256j+128h+q]
    h0t = _bf16(
        np.ascontiguousarray(h0.reshape(B, 4, 2, 128).transpose(3, 2, 1, 0).reshape(128, 256))
    )
    embT = emb_w.T                                        # [H, V]
    NV = Vs // 500
    maps = []
    for c in range(n_cores):
        embk = _kblock(np.ascontiguousarray(embT[:, c * Vs : (c + 1) * Vs]))
        # n-major: chunk n's 8 k-slices contiguous -> [p, (n k v)]
        embc = _bf16(
            np.ascontiguousarray(
                embk.reshape(128, NK, NV, 500).transpose(0, 2, 1, 3).reshape(128, NK * Vs)
            )
        )
        maps.append(
            dict(
                whhp=whhp, embc=embc, gistk=gistk, bhhn=bhhn,
                i128=i128, h0b=h0b, h0t=h0t,
            )
        )
    return maps


_CACHE = {}


def run(enc_hiddens, emb_w, w_ih, w_hh, b_ih, b_hh, gold, T, Vs, n_cores, Tc,
        trace=False):
    key = (T, Vs, n_cores, Tc)
    if key not in _CACHE:
        _CACHE[key] = build_program(T, Vs, Tc)
    nc = _CACHE[key]
    maps = prep_inputs(enc_hiddens, emb_w, w_ih, w_hh, b_ih, b_hh, gold, T, Vs, n_cores)
    res = run_bass_kernel_spmd(nc, maps, list(range(n_cores)), trace=trace)
    out = np.concatenate(
        [res.results[c]["scores"].astype(np.float32) for c in range(n_cores)], axis=2
    )
    return out, res


def kernel(enc_hiddens, emb_w, w_ih, w_hh, b_ih, b_hh, gold):
    T, Vs = 256, 32000 // N_CORES
    out, _ = run(enc_hiddens, emb_w, w_ih, w_hh, b_ih, b_hh, gold, T, Vs, N_CORES, Tc=8)
    return out
